# revision 1
# baseline (speedup 1.0000x reference)
import numpy as np
import jax
import jax.numpy as jnp

# nn_DANet_Attention: B=8, C=128, T=8, H=W=32.
# Sharding: pure data parallelism over batch B across the 8 NeuronCores
# (one sample per core); conv weights and gammas replicated.

B, C, T, H, W = 8, 128, 8, 32, 32
HW = H * W


def _danet_one(x, Wq, bq, Wk, bk, Wv, bv, gamma_pam, gamma_cam, gamma_tim):
    # x: [C, T, H, W] (single sample, per core)
    q = jnp.einsum('cthw,oc->othw', x, Wq) + bq[:, None, None, None]
    k = jnp.einsum('cthw,oc->othw', x, Wk) + bk[:, None, None, None]
    v = jnp.einsum('cthw,oc->othw', x, Wv) + bv[:, None, None, None]
    qf = q.reshape(-1, HW)           # [(C//8)*T, HW]
    kf = k.reshape(-1, HW)
    vf = v.reshape(-1, HW)           # [C*T, HW]
    energy = qf.T @ kf               # [HW, HW]
    attn = jax.nn.softmax(energy, axis=-1)
    pam = (vf @ attn.T).reshape(C, T, H, W)
    pam = gamma_pam[0] * pam + x

    xc = x.reshape(C, -1)            # [C, T*HW]
    e = xc @ xc.T                    # [C, C]
    e = jnp.max(e, axis=-1, keepdims=True) - e
    a = jax.nn.softmax(e, axis=-1)
    cam = (a @ xc).reshape(C, T, H, W)
    cam = gamma_cam[0] * cam + x

    xt = jnp.transpose(x, (1, 0, 2, 3)).reshape(T, -1)  # [T, C*HW]
    e = xt @ xt.T                    # [T, T]
    e = jnp.max(e, axis=-1, keepdims=True) - e
    a = jax.nn.softmax(e, axis=-1)
    tim = (a @ xt).reshape(T, C, H, W)
    tim = jnp.transpose(tim, (1, 0, 2, 3))
    tim = gamma_tim[0] * tim + x

    return pam + cam + tim


_pmapped = jax.pmap(
    _danet_one,
    in_axes=(0, None, None, None, None, None, None, None, None, None),
)


def kernel(x, Wq, bq, Wk, bk, Wv, bv, gamma_pam, gamma_cam, gamma_tim):
    x = jnp.asarray(x, dtype=jnp.float32)
    args = [jnp.asarray(a, dtype=jnp.float32)
            for a in (Wq, bq, Wk, bk, Wv, bv, gamma_pam, gamma_cam, gamma_tim)]
    n_dev = jax.local_device_count()
    if n_dev >= B:
        out = _pmapped(x, *args)
    else:
        # Fallback: run on a single device
        out = jax.vmap(_danet_one, in_axes=(0,) + (None,) * 9)(x, *args)
    return np.asarray(out, dtype=np.float32)


# revision 2
# speedup vs baseline: 1.0340x; 1.0340x over previous
import numpy as np
import jax
import jax.numpy as jnp

# nn_DANet_Attention: B=8, C=128, T=8, H=W=32.
# Sharding: pure data parallelism over batch B across the 8 NeuronCores
# (one sample per core); conv weights and gammas replicated.

B, C, T, H, W = 8, 128, 8, 32, 32
HW = H * W


def _danet_one(x, Wq, bq, Wk, bk, Wv, bv, gamma_pam, gamma_cam, gamma_tim):
    # x: [C, T, H, W] (single sample, per core)
    q = jnp.einsum('cthw,oc->othw', x, Wq) + bq[:, None, None, None]
    k = jnp.einsum('cthw,oc->othw', x, Wk) + bk[:, None, None, None]
    v = jnp.einsum('cthw,oc->othw', x, Wv) + bv[:, None, None, None]
    qf = q.reshape(-1, HW)           # [(C//8)*T, HW]
    kf = k.reshape(-1, HW)
    vf = v.reshape(-1, HW)           # [C*T, HW]
    energy = qf.T @ kf               # [HW, HW]
    attn = jax.nn.softmax(energy, axis=-1)
    pam = (vf @ attn.T).reshape(C, T, H, W)
    pam = gamma_pam[0] * pam + x

    xc = x.reshape(C, -1)            # [C, T*HW]
    e = xc @ xc.T                    # [C, C]
    e = jnp.max(e, axis=-1, keepdims=True) - e
    a = jax.nn.softmax(e, axis=-1)
    cam = (a @ xc).reshape(C, T, H, W)
    cam = gamma_cam[0] * cam + x

    xt = jnp.transpose(x, (1, 0, 2, 3)).reshape(T, -1)  # [T, C*HW]
    e = xt @ xt.T                    # [T, T]
    e = jnp.max(e, axis=-1, keepdims=True) - e
    a = jax.nn.softmax(e, axis=-1)
    tim = (a @ xt).reshape(T, C, H, W)
    tim = jnp.transpose(tim, (1, 0, 2, 3))
    tim = gamma_tim[0] * tim + x

    return pam + cam + tim


_pmapped = jax.pmap(
    _danet_one,
    in_axes=(0, None, None, None, None, None, None, None, None, None),
)


def kernel(x, Wq, bq, Wk, bk, Wv, bv, gamma_pam, gamma_cam, gamma_tim):
    x = np.asarray(x, dtype=np.float32)
    args = [np.asarray(a, dtype=np.float32)
            for a in (Wq, bq, Wk, bk, Wv, bv, gamma_pam, gamma_cam, gamma_tim)]
    n_dev = jax.local_device_count()
    if n_dev >= B:
        out = _pmapped(x, *args)
    else:
        # Fallback: run on a single device
        out = jax.vmap(_danet_one, in_axes=(0,) + (None,) * 9)(x, *args)
    return np.asarray(out, dtype=np.float32)
